# revision 1
# baseline (speedup 1.0000x reference)
"""Trainium2 Bass kernel for nn_CondAttentionTemporalModule.

Sharding: data-parallel over the b*(h*w)=2048 attention batch -> 256 seqs/core
on 8 NeuronCores. The FLOP-dominant dense projections (Q/K/V and output
projection, ~97% of FLOPs) run on-device as bf16 matmuls with fp32 accumulate
and fp32 residual adds; tiny per-sequence glue (layernorm stats, RoPE twiddle,
16x16 softmax) runs on host in numpy.
"""
import numpy as np

import concourse.bacc as bacc
import concourse.mybir as mybir
import concourse.tile as tile
from concourse.bass_utils import run_bass_kernel_spmd

N_CORES = 8
B, C, T, H, W = 2, 256, 16, 32, 32
HEADS, DHEAD = 8, 32
SEQS = B * H * W            # 2048
S_CORE = SEQS // N_CORES    # 256 seqs per core
TOK = S_CORE * T            # 4096 tokens per core
EPS = 1e-5

_cache = {}


def _build_qkv():
    # Y[4096,768] = [Xq@Wq | Xkv@Wk | Xkv@Wv]; inputs pre-transposed/packed.
    nc = bacc.Bacc("TRN2", target_bir_lowering=False, debug=False,
                   num_devices=N_CORES)
    bf16 = mybir.dt.bfloat16
    f32 = mybir.dt.float32
    xq_d = nc.dram_tensor("xq", (128, 2 * TOK), bf16, kind="ExternalInput")
    xkv_d = nc.dram_tensor("xkv", (128, 2 * TOK), bf16, kind="ExternalInput")
    w_d = nc.dram_tensor("w", (128, 2, 768), bf16, kind="ExternalInput")
    y_d = nc.dram_tensor("y", (TOK, 768), f32, kind="ExternalOutput")
    with tile.TileContext(nc) as tc:
        with (
            tc.tile_pool(name="consts", bufs=1) as consts,
            tc.tile_pool(name="io", bufs=3) as io,
            tc.tile_pool(name="ps", bufs=4, space="PSUM") as ps,
        ):
            xq = consts.tile([128, 2 * TOK], bf16)
            nc.sync.dma_start(xq[:], xq_d.ap())
            xkv = consts.tile([128, 2 * TOK], bf16)
            nc.sync.dma_start(xkv[:], xkv_d.ap())
            w = consts.tile([128, 2, 768], bf16)
            nc.sync.dma_start(w[:], w_d.ap())
            for t in range(TOK // 128):
                for nb in range(3):
                    src = xq if nb == 0 else xkv
                    acc = ps.tile([128, 256], f32)
                    for kc in range(2):
                        nc.tensor.matmul(
                            acc[:],
                            src[:, kc * TOK + t * 128: kc * TOK + t * 128 + 128],
                            w[:, kc, nb * 256:(nb + 1) * 256],
                            start=(kc == 0), stop=(kc == 1),
                        )
                    o = io.tile([128, 256], f32)
                    nc.vector.tensor_copy(o[:], acc[:])
                    nc.sync.dma_start(
                        y_d.ap()[t * 128:(t + 1) * 128, nb * 256:(nb + 1) * 256],
                        o[:])
    nc.compile()
    return nc


def _build_oproj():
    # Y[4096,256] = X@Wo + R
    nc = bacc.Bacc("TRN2", target_bir_lowering=False, debug=False,
                   num_devices=N_CORES)
    bf16 = mybir.dt.bfloat16
    f32 = mybir.dt.float32
    x_d = nc.dram_tensor("x", (128, 2 * TOK), bf16, kind="ExternalInput")
    w_d = nc.dram_tensor("w", (128, 2, 256), bf16, kind="ExternalInput")
    r_d = nc.dram_tensor("r", (TOK, 256), f32, kind="ExternalInput")
    y_d = nc.dram_tensor("y", (TOK, 256), f32, kind="ExternalOutput")
    with tile.TileContext(nc) as tc:
        with (
            tc.tile_pool(name="consts", bufs=1) as consts,
            tc.tile_pool(name="io", bufs=3) as io,
            tc.tile_pool(name="ps", bufs=4, space="PSUM") as ps,
        ):
            x = consts.tile([128, 2 * TOK], bf16)
            nc.sync.dma_start(x[:], x_d.ap())
            w = consts.tile([128, 2, 256], bf16)
            nc.sync.dma_start(w[:], w_d.ap())
            for t in range(TOK // 128):
                acc = ps.tile([128, 256], f32)
                for kc in range(2):
                    nc.tensor.matmul(
                        acc[:],
                        x[:, kc * TOK + t * 128: kc * TOK + t * 128 + 128],
                        w[:, kc, :],
                        start=(kc == 0), stop=(kc == 1),
                    )
                r = io.tile([128, 256], f32)
                nc.sync.dma_start(r[:], r_d.ap()[t * 128:(t + 1) * 128, :])
                o = io.tile([128, 256], f32)
                nc.vector.tensor_add(o[:], acc[:], r[:])
                nc.sync.dma_start(y_d.ap()[t * 128:(t + 1) * 128, :], o[:])
    nc.compile()
    return nc


def _pack_xt(x):
    # [TOK, 256] fp32 -> bf16 [128, 2*TOK] (two 128-channel chunks side by side)
    xt = np.ascontiguousarray(x.T.astype(np.bfloat16) if hasattr(np, "bfloat16")
                              else x.T)
    return np.concatenate([xt[:128], xt[128:]], axis=1)


def _to_bf16(a):
    import ml_dtypes
    return a.astype(ml_dtypes.bfloat16)


def _pack_xt2(x):
    xt = _to_bf16(np.ascontiguousarray(x.T))
    return np.ascontiguousarray(np.concatenate([xt[:128], xt[128:]], axis=1))


def _pack_w(w):
    # [256, N] -> bf16 [128, 2, N]
    return np.ascontiguousarray(
        np.stack([_to_bf16(w[:128]), _to_bf16(w[128:])], axis=1))


def _ln(x, g, b):
    mu = x.mean(-1, keepdims=True)
    var = x.var(-1, keepdims=True)
    return (x - mu) / np.sqrt(var + EPS) * g + b


def _rope(x):
    # x: [S, HEADS, T, DHEAD] -> rotary over T axis, interleaved pairs
    n, d = T, DHEAD
    inv = 1.0 / (10000.0 ** (np.arange(0, d, 2, dtype=np.float32) / d))
    ang = np.arange(n, dtype=np.float32)[:, None] * inv[None, :]
    ang = np.repeat(ang, 2, axis=-1)
    cos, sin = np.cos(ang), np.sin(ang)
    xp = x.reshape(x.shape[:-1] + (d // 2, 2))
    rot = np.stack((-xp[..., 1], xp[..., 0]), axis=-1).reshape(x.shape)
    return x * cos + rot * sin


def _run(nc, in_maps):
    return run_bass_kernel_spmd(nc, in_maps, list(range(N_CORES))).results


def _qkv_device(xq_cores, xkv_cores, wq, wk, wv):
    nc = _cache.setdefault("qkv", None) or _cache.setdefault("qkv_b", None)
    if _cache.get("qkv_nc") is None:
        _cache["qkv_nc"] = _build_qkv()
    nc = _cache["qkv_nc"]
    wpk = _pack_w(np.concatenate([wq, wk, wv], axis=1))
    maps = [{"xq": _pack_xt2(xq_cores[i]), "xkv": _pack_xt2(xkv_cores[i]),
             "w": wpk} for i in range(N_CORES)]
    res = _run(nc, maps)
    return [r["y"] for r in res]


def _oproj_device(x_cores, wo, r_cores):
    if _cache.get("oproj_nc") is None:
        _cache["oproj_nc"] = _build_oproj()
    nc = _cache["oproj_nc"]
    wpk = _pack_w(wo)
    maps = [{"x": _pack_xt2(x_cores[i]), "w": wpk,
             "r": np.ascontiguousarray(r_cores[i], dtype=np.float32)}
            for i in range(N_CORES)]
    res = _run(nc, maps)
    return [r["y"] for r in res]


def _attention(qkv_cores, pos_bias):
    # qkv: per core [TOK, 768] fp32 -> attn out [TOK, 256]
    outs = []
    scale = DHEAD ** -0.5
    pb = pos_bias[0]  # [HEADS, T, T]
    for y in qkv_cores:
        y = y.reshape(S_CORE, T, 3, HEADS, DHEAD)  # wait: cols = [q|k|v] 256 each
        outs.append(y)
    res = []
    for y in qkv_cores:
        q = y[:, 0:256].reshape(S_CORE, T, HEADS, DHEAD).transpose(0, 2, 1, 3)
        k = y[:, 256:512].reshape(S_CORE, T, HEADS, DHEAD).transpose(0, 2, 1, 3)
        v = y[:, 512:768].reshape(S_CORE, T, HEADS, DHEAD).transpose(0, 2, 1, 3)
        q = _rope(q * scale)
        k = _rope(k)
        sim = np.einsum("shid,shjd->shij", q, k) + pb[None]
        sim = sim - sim.max(-1, keepdims=True)
        e = np.exp(sim)
        a = e / e.sum(-1, keepdims=True)
        o = np.einsum("shij,shjd->shid", a, v)          # [S, H, T, D]
        o = o.transpose(0, 2, 1, 3).reshape(TOK, 256)
        res.append(np.ascontiguousarray(o, dtype=np.float32))
    return res


def kernel(x, motion_map, pos_bias, g1, b1, Wq1, Wk1, Wv1, Wo1,
           g2, b2, cg, cb, Wq2, Wk2, Wv2, Wo2):
    f = np.asarray
    x = f(x, dtype=np.float32)
    motion_map = f(motion_map, dtype=np.float32)
    xs = x.transpose(0, 3, 4, 2, 1).reshape(SEQS, T, C)
    mm = motion_map.transpose(0, 3, 4, 2, 1).reshape(SEQS, T, C)
    xs_c = [np.ascontiguousarray(xs[i * S_CORE:(i + 1) * S_CORE].reshape(TOK, C))
            for i in range(N_CORES)]
    mm_c = [np.ascontiguousarray(mm[i * S_CORE:(i + 1) * S_CORE].reshape(TOK, C))
            for i in range(N_CORES)]

    # layer 1: self attention
    xn1 = [_ln(a, f(g1), f(b1)) for a in xs_c]
    qkv1 = _qkv_device(xn1, xn1, f(Wq1), f(Wk1), f(Wv1))
    at1 = _attention(qkv1, f(pos_bias))
    xs1 = _oproj_device(at1, f(Wo1), xs_c)

    # layer 2: cross attention with motion map
    xn2 = [_ln(a, f(g2), f(b2)) for a in xs1]
    ctx = [_ln(a, f(cg), f(cb)) for a in mm_c]
    qkv2 = _qkv_device(xn2, ctx, f(Wq2), f(Wk2), f(Wv2))
    at2 = _attention(qkv2, f(pos_bias))
    xs2 = _oproj_device(at2, f(Wo2), xs1)

    out = np.concatenate([a.reshape(S_CORE, T, C) for a in xs2], axis=0)
    out = out.reshape(B, H, W, T, C).transpose(0, 4, 3, 1, 2)
    return np.ascontiguousarray(out, dtype=np.float32)


# revision 3
# speedup vs baseline: 1.1601x; 1.1601x over previous
"""Trainium2 Bass kernel for nn_CondAttentionTemporalModule.

Sharding: data-parallel over the b*(h*w)=2048 attention batch -> 256 seqs/core
on 8 NeuronCores. The FLOP-dominant dense projections (Q/K/V and output
projection, ~97% of FLOPs) run on-device as bf16 matmuls with fp32 accumulate
and fp32 residual adds; tiny per-sequence glue (layernorm stats, RoPE twiddle,
16x16 softmax) runs on host in numpy.
"""
import numpy as np

import concourse.bacc as bacc
import concourse.mybir as mybir
import concourse.tile as tile
from concourse.bass_utils import run_bass_kernel_spmd

N_CORES = 8
B, C, T, H, W = 2, 256, 16, 32, 32
HEADS, DHEAD = 8, 32
SEQS = B * H * W            # 2048
S_CORE = SEQS // N_CORES    # 256 seqs per core
TOK = S_CORE * T            # 4096 tokens per core
EPS = 1e-5

_cache = {}


def _build_qkv():
    # Y[4096,768] = [Xq@Wq | Xkv@Wk | Xkv@Wv]; inputs pre-transposed/packed.
    nc = bacc.Bacc("TRN2", target_bir_lowering=False, debug=False,
                   num_devices=N_CORES)
    bf16 = mybir.dt.bfloat16
    f32 = mybir.dt.float32
    xq_d = nc.dram_tensor("xq", (128, 2 * TOK), bf16, kind="ExternalInput")
    xkv_d = nc.dram_tensor("xkv", (128, 2 * TOK), bf16, kind="ExternalInput")
    w_d = nc.dram_tensor("w", (128, 2, 768), bf16, kind="ExternalInput")
    y_d = nc.dram_tensor("y", (TOK, 768), f32, kind="ExternalOutput")
    with tile.TileContext(nc) as tc:
        with (
            tc.tile_pool(name="consts", bufs=1) as consts,
            tc.tile_pool(name="io", bufs=3) as io,
            tc.tile_pool(name="ps", bufs=4, space="PSUM") as ps,
        ):
            xq = consts.tile([128, 2 * TOK], bf16)
            nc.sync.dma_start(xq[:], xq_d.ap())
            xkv = consts.tile([128, 2 * TOK], bf16)
            nc.sync.dma_start(xkv[:], xkv_d.ap())
            w = consts.tile([128, 2, 768], bf16)
            nc.sync.dma_start(w[:], w_d.ap())
            for t in range(TOK // 128):
                for nb in range(3):
                    src = xq if nb == 0 else xkv
                    acc = ps.tile([128, 256], f32)
                    for kc in range(2):
                        nc.tensor.matmul(
                            acc[:],
                            src[:, kc * TOK + t * 128: kc * TOK + t * 128 + 128],
                            w[:, kc, nb * 256:(nb + 1) * 256],
                            start=(kc == 0), stop=(kc == 1),
                        )
                    o = io.tile([128, 256], f32)
                    nc.vector.tensor_copy(o[:], acc[:])
                    nc.sync.dma_start(
                        y_d.ap()[t * 128:(t + 1) * 128, nb * 256:(nb + 1) * 256],
                        o[:])
    nc.compile()
    return nc


def _build_oproj():
    # Y[4096,256] = X@Wo + R
    nc = bacc.Bacc("TRN2", target_bir_lowering=False, debug=False,
                   num_devices=N_CORES)
    bf16 = mybir.dt.bfloat16
    f32 = mybir.dt.float32
    x_d = nc.dram_tensor("x", (128, 2 * TOK), bf16, kind="ExternalInput")
    w_d = nc.dram_tensor("w", (128, 2, 256), bf16, kind="ExternalInput")
    r_d = nc.dram_tensor("r", (TOK, 256), f32, kind="ExternalInput")
    y_d = nc.dram_tensor("y", (TOK, 256), f32, kind="ExternalOutput")
    with tile.TileContext(nc) as tc:
        with (
            tc.tile_pool(name="consts", bufs=1) as consts,
            tc.tile_pool(name="io", bufs=3) as io,
            tc.tile_pool(name="ps", bufs=4, space="PSUM") as ps,
        ):
            x = consts.tile([128, 2 * TOK], bf16)
            nc.sync.dma_start(x[:], x_d.ap())
            w = consts.tile([128, 2, 256], bf16)
            nc.sync.dma_start(w[:], w_d.ap())
            for t in range(TOK // 128):
                acc = ps.tile([128, 256], f32)
                for kc in range(2):
                    nc.tensor.matmul(
                        acc[:],
                        x[:, kc * TOK + t * 128: kc * TOK + t * 128 + 128],
                        w[:, kc, :],
                        start=(kc == 0), stop=(kc == 1),
                    )
                r = io.tile([128, 256], f32)
                nc.sync.dma_start(r[:], r_d.ap()[t * 128:(t + 1) * 128, :])
                o = io.tile([128, 256], f32)
                nc.vector.tensor_add(o[:], acc[:], r[:])
                nc.sync.dma_start(y_d.ap()[t * 128:(t + 1) * 128, :], o[:])
    nc.compile()
    return nc


def _pack_xt(x):
    # [TOK, 256] fp32 -> bf16 [128, 2*TOK] (two 128-channel chunks side by side)
    xt = np.ascontiguousarray(x.T.astype(np.bfloat16) if hasattr(np, "bfloat16")
                              else x.T)
    return np.concatenate([xt[:128], xt[128:]], axis=1)


def _to_bf16(a):
    import ml_dtypes
    return a.astype(ml_dtypes.bfloat16)


def _pack_xt2(x):
    xt = _to_bf16(np.ascontiguousarray(x.T))
    return np.ascontiguousarray(np.concatenate([xt[:128], xt[128:]], axis=1))


def _pack_w(w):
    # [256, N] -> bf16 [128, 2, N]
    return np.ascontiguousarray(
        np.stack([_to_bf16(w[:128]), _to_bf16(w[128:])], axis=1))


def _ln(x, g, b):
    mu = x.mean(-1, keepdims=True)
    var = x.var(-1, keepdims=True)
    return (x - mu) / np.sqrt(var + EPS) * g + b


def _rope(x):
    # x: [S, HEADS, T, DHEAD] -> rotary over T axis, interleaved pairs
    n, d = T, DHEAD
    inv = 1.0 / (10000.0 ** (np.arange(0, d, 2, dtype=np.float32) / d))
    ang = np.arange(n, dtype=np.float32)[:, None] * inv[None, :]
    ang = np.repeat(ang, 2, axis=-1)
    cos, sin = np.cos(ang), np.sin(ang)
    xp = x.reshape(x.shape[:-1] + (d // 2, 2))
    rot = np.stack((-xp[..., 1], xp[..., 0]), axis=-1).reshape(x.shape)
    return x * cos + rot * sin


def _make_runner(nc):
    # cached equivalent of bass2jax.run_bass_via_pjrt: build the jitted
    # shard_map executable ONCE so steady-state calls skip retracing.
    import jax
    from concourse.bass2jax import (_bass_exec_p, install_neuronx_cc_hook,
                                    Mesh, PartitionSpec, shard_map)
    install_neuronx_cc_hook()
    in_names, out_names, out_avals = [], [], []
    for alloc in nc.m.functions[0].allocations:
        if not isinstance(alloc, mybir.MemoryLocationSet):
            continue
        name = alloc.memorylocations[0].name
        if alloc.kind == "ExternalInput":
            in_names.append(name)
        elif alloc.kind == "ExternalOutput":
            out_names.append(name)
            out_avals.append(jax.core.ShapedArray(
                tuple(alloc.tensor_shape), mybir.dt.np(alloc.dtype)))
    pname = nc.partition_id_tensor.name if nc.partition_id_tensor else None
    if pname is not None and pname in in_names:
        in_names.remove(pname)
    n_params, n_outs = len(in_names), len(out_names)
    all_in = tuple(in_names + out_names) + ((pname,) if pname else ())

    def _body(*args):
        operands = list(args)
        if pname is not None:
            from concourse.bass2jax import partition_id_tensor
            operands.append(partition_id_tensor())
        return tuple(_bass_exec_p.bind(
            *operands, out_avals=tuple(out_avals), in_names=all_in,
            out_names=tuple(out_names), lowering_input_output_aliases=(),
            sim_require_finite=True, sim_require_nnan=True, nc=nc))

    mesh = Mesh(np.asarray(jax.devices()[:N_CORES]), ("core",))
    sharded = jax.jit(
        shard_map(_body, mesh=mesh,
                  in_specs=(PartitionSpec("core"),) * (n_params + n_outs),
                  out_specs=(PartitionSpec("core"),) * n_outs,
                  check_rep=False),
        donate_argnums=tuple(range(n_params, n_params + n_outs)),
        keep_unused=True)

    def run(in_maps):
        concat_in = [np.concatenate([np.asarray(m[nm]) for m in in_maps],
                                    axis=0) for nm in in_names]
        concat_zeros = [np.zeros((N_CORES * a.shape[0], *a.shape[1:]), a.dtype)
                        for a in out_avals]
        outs = sharded(*concat_in, *concat_zeros)
        return [{nm: np.asarray(outs[i]).reshape(N_CORES, *out_avals[i].shape)[c]
                 for i, nm in enumerate(out_names)} for c in range(N_CORES)]

    return run


def _run(nc, in_maps):
    key = id(nc)
    if key not in _cache:
        _cache[key] = _make_runner(nc)
    return _cache[key](in_maps)


def _qkv_device(xq_cores, xkv_cores, wq, wk, wv):
    nc = _cache.setdefault("qkv", None) or _cache.setdefault("qkv_b", None)
    if _cache.get("qkv_nc") is None:
        _cache["qkv_nc"] = _build_qkv()
    nc = _cache["qkv_nc"]
    wpk = _pack_w(np.concatenate([wq, wk, wv], axis=1))
    maps = [{"xq": _pack_xt2(xq_cores[i]), "xkv": _pack_xt2(xkv_cores[i]),
             "w": wpk} for i in range(N_CORES)]
    res = _run(nc, maps)
    return [r["y"] for r in res]


def _oproj_device(x_cores, wo, r_cores):
    if _cache.get("oproj_nc") is None:
        _cache["oproj_nc"] = _build_oproj()
    nc = _cache["oproj_nc"]
    wpk = _pack_w(wo)
    maps = [{"x": _pack_xt2(x_cores[i]), "w": wpk,
             "r": np.ascontiguousarray(r_cores[i], dtype=np.float32)}
            for i in range(N_CORES)]
    res = _run(nc, maps)
    return [r["y"] for r in res]


def _attention(qkv_cores, pos_bias):
    # qkv: per core [TOK, 768] fp32 -> attn out [TOK, 256]
    outs = []
    scale = DHEAD ** -0.5
    pb = pos_bias[0]  # [HEADS, T, T]
    for y in qkv_cores:
        y = y.reshape(S_CORE, T, 3, HEADS, DHEAD)  # wait: cols = [q|k|v] 256 each
        outs.append(y)
    res = []
    for y in qkv_cores:
        q = y[:, 0:256].reshape(S_CORE, T, HEADS, DHEAD).transpose(0, 2, 1, 3)
        k = y[:, 256:512].reshape(S_CORE, T, HEADS, DHEAD).transpose(0, 2, 1, 3)
        v = y[:, 512:768].reshape(S_CORE, T, HEADS, DHEAD).transpose(0, 2, 1, 3)
        q = _rope(q * scale)
        k = _rope(k)
        sim = np.einsum("shid,shjd->shij", q, k) + pb[None]
        sim = sim - sim.max(-1, keepdims=True)
        e = np.exp(sim)
        a = e / e.sum(-1, keepdims=True)
        o = np.einsum("shij,shjd->shid", a, v)          # [S, H, T, D]
        o = o.transpose(0, 2, 1, 3).reshape(TOK, 256)
        res.append(np.ascontiguousarray(o, dtype=np.float32))
    return res


def kernel(x, motion_map, pos_bias, g1, b1, Wq1, Wk1, Wv1, Wo1,
           g2, b2, cg, cb, Wq2, Wk2, Wv2, Wo2):
    f = np.asarray
    x = f(x, dtype=np.float32)
    motion_map = f(motion_map, dtype=np.float32)
    xs = x.transpose(0, 3, 4, 2, 1).reshape(SEQS, T, C)
    mm = motion_map.transpose(0, 3, 4, 2, 1).reshape(SEQS, T, C)
    xs_c = [np.ascontiguousarray(xs[i * S_CORE:(i + 1) * S_CORE].reshape(TOK, C))
            for i in range(N_CORES)]
    mm_c = [np.ascontiguousarray(mm[i * S_CORE:(i + 1) * S_CORE].reshape(TOK, C))
            for i in range(N_CORES)]

    # layer 1: self attention
    xn1 = [_ln(a, f(g1), f(b1)) for a in xs_c]
    qkv1 = _qkv_device(xn1, xn1, f(Wq1), f(Wk1), f(Wv1))
    at1 = _attention(qkv1, f(pos_bias))
    xs1 = _oproj_device(at1, f(Wo1), xs_c)

    # layer 2: cross attention with motion map
    xn2 = [_ln(a, f(g2), f(b2)) for a in xs1]
    ctx = [_ln(a, f(cg), f(cb)) for a in mm_c]
    qkv2 = _qkv_device(xn2, ctx, f(Wq2), f(Wk2), f(Wv2))
    at2 = _attention(qkv2, f(pos_bias))
    xs2 = _oproj_device(at2, f(Wo2), xs1)

    out = np.concatenate([a.reshape(S_CORE, T, C) for a in xs2], axis=0)
    out = out.reshape(B, H, W, T, C).transpose(0, 4, 3, 1, 2)
    return np.ascontiguousarray(out, dtype=np.float32)
